# revision 3
# baseline (speedup 1.0000x reference)
"""Multi-head attention (B=2,S=2048,D=1024,H=16,A=64) on 8 trn2 NeuronCores.

Sharding: core = 4*b + g (b = batch, g = head-group of 4 heads = 2 pairs).

v2 design ("exp split + dense PE streams"):
- Wq is pre-scaled host-side by 16*log2(e), so scores arrive in "z2" units
  (z2 = 128 * log2(exp_arg)).  exp(score/8) == 2^(z2/128).
- exp work is split across TWO engines:
    ACT:  activation(Exp, scale=ln2/128) -> pt bf16            (exact)
    DVE:  custom op EXP_BITS_ANT: Schraudolph-with-quadratic-
          correction computes the bf16 BIT PATTERN of 2^(z2/128) as an
          f32 value, converted to int16 (round) whose bytes ARE bf16.
          (~1.8% max per-weight err; end-to-end sim rel err 0.0074.)
- scores: K=64 matmuls for the two heads of a pair use disjoint row
  halves (partitions 0:64 / 64:128) and are emitted adjacently -> the
  PE runs them CONCURRENTLY (measured pair gap ~3ns) and LDWEIGHTS
  hides behind the partner stream.
- all other matmuls are emitted as dense per-stage streams so LDWEIGHTS
  prefetches into the background weight buffer (measured: alternating
  weights at N=512 sustains 216ns/MM warm).
- AV with a ones-column in v (M=65) produces the softmax denominator;
  normalize via approx-reciprocal + gpsimd partition broadcast.
- fc_out psum->SBUF copies are split between the scalar and vector
  engines; bo is added on the host during the gather (zero HW cost).
"""

import numpy as np

B, S, D, H, A = 2, 2048, 1024, 16, 64
GROUPS = 4              # head groups (cores per batch)
HPG = H // GROUPS       # heads per core = 4
C = HPG * A             # channels per core = 256
N_CORES = 8
KD = D // 128           # d-tiles = 8
MC = C // 128           # channel tiles per core = 2 (one per head pair)
NS = S // 128           # seq tiles = 16
QC = 4                  # q chunks
QW = S // QC            # 512
NG = NS // 2            # groups per chunk (2 k-tiles per group)

# DVE exp-op constants (i16/bf16-bits path, z2 units)
K_MAGIC = float(1.5 * 2**30)
EXP_C1 = -0.00269
EXP_C2 = 16253.2
PRESCALE = float(16.0 / np.log(2.0))   # 128*log2(e)/8
ACT_SCALE = float(np.log(2.0) / 128.0)

# exp-unit engine assignment: unit index u = 2*ng + hh in [0, 16) per chunk.
# DVE units chosen to spread; tune ratio for ACT/DVE balance.
import os as _os
if _os.environ.get("K2DVE") == "0":
    DVE_UNITS = set()
elif _os.environ.get("K2DVE"):
    DVE_UNITS = {int(u) for u in _os.environ["K2DVE"].split(",")}
else:
    DVE_UNITS = {1, 4, 7, 10, 13}      # 5 of 16 per chunk -> 40 of 128
USE_SCALAR_COPY = _os.environ.get("K2SC", "1") == "1"


def register_exp_op():
    import concourse.dve_ops as dve_ops

    for op in dve_ops.OPS:
        if op.name == "EXP_BITS_ANT":
            return op
    from concourse.dve_spec import Spec, Src0, C0, C1, C2, relu, sq, lower
    from concourse.dve_spec import _has_src1 as has_src1
    from concourse.dve_uop import DveOpSpec

    u = Src0 + C0
    n2 = u - C0
    f2 = Src0 - n2
    body = relu(Src0 + sq(f2) * C1 + C2)

    def ref(in0, in1, s0, s1, imm2):
        uu = (in0 + np.float32(s0)).astype(np.float32)
        nn = (uu - np.float32(s0)).astype(np.float32)
        ff = (in0 - nn).astype(np.float32)
        return np.maximum(in0 + ff * ff * np.float32(s1) + np.float32(imm2),
                          0).astype(np.float32)

    spec = Spec(body=body, reference=ref)
    shas = {}
    for ver in ("v3", "v4"):
        s = DveOpSpec(name="EXP_BITS_ANT", opcode=1, uops=lower(spec, ver=ver),
                      rd1_en=has_src1(spec))
        shas[ver] = s.sha(ver)
    op = dve_ops.DveOp("EXP_BITS_ANT", spec, subdim=False, uops_sha=shas)
    dve_ops.OPS.append(op)
    dve_ops._SUB_OPCODE_FOR_NAME[op.name] = (
        dve_ops._CUSTOM_DVE_ROW_BASE + len(dve_ops.OPS) - 1)
    assert dve_ops._SUB_OPCODE_FOR_NAME[op.name] < 0x20
    return op


def build_nc():
    import concourse.mybir as mybir
    import concourse.tile as tile
    from concourse import bacc

    EXP_OP = register_exp_op()
    f32 = mybir.dt.float32
    bf16 = mybir.dt.bfloat16
    i16 = mybir.dt.int16
    AF = mybir.ActivationFunctionType

    nc = bacc.Bacc(
        "TRN2", target_bir_lowering=False, debug=False,
        enable_asserts=True, num_devices=N_CORES,
    )

    xT_d = nc.dram_tensor("xT", [D, S], bf16, kind="ExternalInput").ap()
    wq_d = nc.dram_tensor("wq", [D, C], bf16, kind="ExternalInput").ap()
    wk_d = nc.dram_tensor("wk", [D, C], bf16, kind="ExternalInput").ap()
    wv_d = nc.dram_tensor("wv", [D, C], bf16, kind="ExternalInput").ap()
    wo_d = nc.dram_tensor("wo", [C, D], bf16, kind="ExternalInput").ap()
    bqs_d = nc.dram_tensor("bqs", [128, MC], f32, kind="ExternalInput").ap()
    bks_d = nc.dram_tensor("bks", [128, MC], f32, kind="ExternalInput").ap()
    bvb_d = nc.dram_tensor("bvb", [128, C], f32, kind="ExternalInput").ap()
    out_d = nc.dram_tensor("out", [S, D], bf16, kind="ExternalOutput").ap()
    dbg = _os.environ.get("K2DBG") == "1"
    if dbg:
        dst_d = nc.dram_tensor("dst", [128, 2, QW], f32,
                               kind="ExternalOutput").ap()
        dpt_d = nc.dram_tensor("dpt", [128, 2, QW], i16,
                               kind="ExternalOutput").ap()

    with tile.TileContext(nc) as tc:
        with tc.tile_pool(name="const", bufs=1) as cpool, \
             tc.tile_pool(name="wgt", bufs=1) as wpool, \
             tc.tile_pool(name="qkv", bufs=1) as qpool, \
             tc.tile_pool(name="ptp", bufs=6) as ptpool, \
             tc.tile_pool(name="rcp", bufs=3) as rpool, \
             tc.tile_pool(name="osb", bufs=4) as opool, \
             tc.tile_pool(name="pst", bufs=2, space="PSUM") as stp, \
             tc.tile_pool(name="pav", bufs=2, space="PSUM") as avp, \
             tc.tile_pool(name="psp", bufs=2, space="PSUM") as psp:

            # ---- ACT table warm-up: issue a tiny Exp before anything else
            warm = cpool.tile([1, 2], f32, name="warm")
            nc.vector.memset(warm[:], 0.0)
            warm2 = cpool.tile([1, 2], bf16, name="warm2")
            nc.scalar.activation(warm2[:], warm[:], AF.Exp, scale=1.0)

            # ---------------- constants + input loads ----------------
            bq_sb = cpool.tile([128, MC], f32, name="bq_sb")
            nc.sync.dma_start(bq_sb[:], bqs_d[:, :])
            bk_sb = cpool.tile([128, MC], f32, name="bk_sb")
            nc.sync.dma_start(bk_sb[:], bks_d[:, :])

            xT_sb = [wpool.tile([128, S], bf16, name=f"xT{kt}")
                     for kt in range(KD)]
            w_sb = {w: [wpool.tile([128, C], bf16, name=f"w{w}{kt}")
                        for kt in range(KD)]
                    for w in ("q", "k", "v")}
            for kt in range(KD):
                ks = slice(kt * 128, (kt + 1) * 128)
                nc.sync.dma_start(xT_sb[kt][:, 0:QW], xT_d[ks, 0:QW])
                nc.scalar.dma_start(w_sb["q"][kt][:], wq_d[ks, :])
                nc.gpsimd.dma_start(w_sb["k"][kt][:], wk_d[ks, :])
            for kt in range(KD):
                ks = slice(kt * 128, (kt + 1) * 128)
                nc.gpsimd.dma_start(w_sb["v"][kt][:], wv_d[ks, :])
            bvb_sb = cpool.tile([128, C], f32, name="bvb_sb")
            nc.gpsimd.dma_start(bvb_sb[:], bvb_d[:, :])
            for qc in (1, 2, 3):
                qs = slice(qc * QW, (qc + 1) * QW)
                for kt in range(KD):
                    ks = slice(kt * 128, (kt + 1) * 128)
                    eng = nc.gpsimd if qc in (1, 3) else nc.sync
                    eng.dma_start(xT_sb[kt][:, qs], xT_d[ks, qs])
            wo_sb = [wpool.tile([128, D], bf16, name=f"wo{kt}")
                     for kt in range(MC)]
            for kt in range(MC):
                nc.gpsimd.dma_start(wo_sb[kt][:],
                                    wo_d[kt * 128:(kt + 1) * 128, :])

            # v padded per head with a ones column: [128, NS, HPG, A+1]
            v_sb = qpool.tile([128, NS, HPG, A + 1], bf16, name="v_sb")
            vones = cpool.tile([128, NS * HPG], f32, name="vones")
            nc.vector.memset(vones[:], 1.0)
            nc.vector.tensor_copy(
                v_sb[:, :, :, A],
                vones[:].rearrange("p (t h) -> p t h", h=HPG))

            qT_sb = [qpool.tile([128, S], bf16, name=f"qT{p}")
                     for p in range(MC)]
            kT_sb = [qpool.tile([128, S], bf16, name=f"kT{p}")
                     for p in range(MC)]
            attn_sb = [qpool.tile([128, S], bf16, name=f"attn{p}")
                       for p in range(MC)]

            # ---------------- work units ----------------
            def qk_unit(mt, wname, qc):
                qs = slice(qc * QW, (qc + 1) * QW)
                ps = psp.tile([128, QW], f32, name="ps", tag="ps")
                for kt in range(KD):
                    nc.tensor.matmul(
                        ps[:],
                        lhsT=w_sb[wname][kt][:, mt * 128:(mt + 1) * 128],
                        rhs=xT_sb[kt][:, qs],
                        start=(kt == 0), stop=(kt == KD - 1),
                    )
                bias = (bq_sb if wname == "q" else bk_sb)[:, mt:mt + 1]
                dst = (qT_sb if wname == "q" else kT_sb)[mt]
                nc.vector.tensor_scalar_add(dst[:, qs], ps[:], bias)

            def v_unit(st):
                psv = psp.tile([128, C], f32, name="psv", tag="ps")
                for kt in range(KD):
                    nc.tensor.matmul(
                        psv[:],
                        lhsT=xT_sb[kt][:, st * 128:(st + 1) * 128],
                        rhs=w_sb["v"][kt][:],
                        start=(kt == 0), stop=(kt == KD - 1),
                    )
                nc.vector.tensor_add(
                    v_sb[:, st, :, 0:A],
                    psv[:].rearrange("p (h a) -> p h a", a=A),
                    bvb_sb[:].rearrange("p (h a) -> p h a", a=A),
                )

            def fc_unit(mt, nn, copy_eng):
                # fc_out: s-tile mt, d-half nn; contraction over C=256
                ps = psp.tile([128, QW], f32, name="psfc", tag="ps")
                for kt in range(MC):
                    nc.tensor.matmul(
                        ps[:],
                        lhsT=attn_sb[kt][:, mt * 128:(mt + 1) * 128],
                        rhs=wo_sb[kt][:, nn * QW:(nn + 1) * QW],
                        start=(kt == 0), stop=(kt == MC - 1),
                    )
                ob = ob_tiles[mt % 8]
                if not USE_SCALAR_COPY:
                    copy_eng = "vector"
                if copy_eng == "scalar":
                    nc.scalar.copy(ob[:, nn * QW:(nn + 1) * QW], ps[:])
                else:
                    nc.vector.tensor_copy(ob[:, nn * QW:(nn + 1) * QW], ps[:])
                if nn == 1:
                    nc.sync.dma_start(out_d[mt * 128:(mt + 1) * 128, :],
                                      ob[:])

            def attention_chunk(p, qc2, fills=None, chunk_idx=[0],
                                qw=None, qcol=None):
                # qc2 indexes 256-wide q chunks (8 per pair).  One exp call
                # per group covers BOTH heads -> the 4 score MMs of the next
                # group share a single dep, so the scheduler keeps them
                # adjacent and the row-disjoint hh pairs co-stream on the PE.
                fills = fills or {}
                heads = (2 * p, 2 * p + 1)
                if qw is None:
                    qw = QW // 2
                if qcol is None:
                    qcol = qc2 * (QW // 2)
                qs = slice(qcol, qcol + qw)
                ci = chunk_idx[0]
                chunk_idx[0] += 1
                # both heads' accumulators share ONE bank (av2), double-
                # buffered: chunk c+1's AVs use the other buffer, so the
                # first AV of a chunk never waits on the previous chunk's
                # normalize reads (chunk-boundary PE stall).
                av2 = avp.tile([A + 1, 2 * qw], f32, name="av2", tag="av2",
                               padded_shape=[128, QW])
                avs = [av2[:, hh * qw:(hh + 1) * qw] for hh in range(2)]
                for ng in range(NG):
                    st = stp.tile([128, 4, qw], f32, name="st", tag="st",
                                  padded_shape=[128, 4, QW // 2])
                    for jj in range(2):
                        kt = 2 * ng + jj
                        for hh in range(2):
                            off = hh * A
                            nc.tensor.matmul(
                                st[:, 2 * hh + jj, :],
                                lhsT=kT_sb[p][off:off + A,
                                              kt * 128:(kt + 1) * 128],
                                rhs=qT_sb[p][off:off + A, qs],
                                start=True, stop=True,
                            )
                    pt = ptpool.tile([128, 4, qw], bf16, name="pt", tag="pt",
                                     padded_shape=[128, 4, QW // 2])
                    # one exp per group (both heads) on ONE engine: finer
                    # splits (per-hh tiles or intra-group engine splits) were
                    # all measured slower -- scheduler adjacency breaks, or
                    # ScalarE/VectorE PSUM contention and per-call overhead
                    # eat the gain.  5/16 of groups go to the DVE exp op.
                    if ((ci * 8 + ng) * 5) % 16 < 5:
                        nc.vector._custom_dve(
                            EXP_OP, out=pt[:].bitcast(i16), in0=st[:],
                            s0=K_MAGIC, s1=EXP_C1, imm2=EXP_C2)
                    else:
                        nc.scalar.activation(pt[:], st[:], AF.Exp,
                                             scale=ACT_SCALE)
                    for fill in fills.get(ng, ()):
                        fill()
                    for jj in range(2):
                        kt = 2 * ng + jj
                        for hh in range(2):
                            # shared bank: only the chunk's FIRST AV MM
                            # bank-clears; per-element has_written turns
                            # later first-writes into overwrites.
                            nc.tensor.matmul(
                                avs[hh][:],
                                lhsT=v_sb[:, kt, heads[hh], :],
                                rhs=pt[:, 2 * hh + jj, :],
                                start=(kt == 0 and hh == 0),
                                stop=(kt == NS - 1 and hh == 1),
                                skip_group_check=True,
                            )
                # normalize: attn[a, q] = av[a, q] * (1 / av[A, q])
                recs = []
                for hh in range(2):
                    den = rpool.tile([1, qw], f32, name="den", tag=f"den{hh}",
                                     padded_shape=[1, QW])
                    nc.vector.tensor_copy(den[:], avs[hh][A:A + 1, :])
                    rec = rpool.tile([1, qw], f32, name="rec", tag=f"rec{hh}",
                                     padded_shape=[1, QW])
                    nc.vector.reciprocal_approx_fast(out=rec[:], in_=den[:])
                    recs.append(rec)
                for hh in range(2):
                    bc = rpool.tile([A, qw], f32, name="bc", tag=f"bc{hh}",
                                    padded_shape=[A, QW])
                    nc.gpsimd.partition_broadcast(bc[:], recs[hh][:])
                    nc.vector.tensor_mul(
                        attn_sb[p][hh * A:(hh + 1) * A, qs],
                        avs[hh][0:A, :], bc[:])

            # ---------------- schedule ----------------
            ob_tiles = [opool.tile([128, D], bf16, name=f"ob{i}", tag=f"ob{i}")
                        for i in range(8)]

            def qk(mt, w, qc):
                return lambda: qk_unit(mt, w, qc)

            def vu(st):
                return lambda: v_unit(st)

            def fc(mt, nn, eng):
                return lambda: fc_unit(mt, nn, eng)

            # NOTE: fill placement is a CORRECTNESS constraint, not just a
            # perf hint: a read emitted before the producing unit gets no
            # RAW dep from the tile framework (first-write tracking is by
            # emission order). k-projections must be emitted before the
            # score group that reads those kT columns; v units before the
            # AV matmuls of their k-tiles; fc(mt) after attn cols are done.
            qk_unit(0, "q", 0)
            qk_unit(0, "k", 0)
            attention_chunk(0, 0, fills={
                0: [vu(0), vu(1)],
                1: [vu(2), vu(3), qk(0, "k", 1)],
                2: [vu(4), vu(5)],
                3: [vu(6), vu(7), qk(0, "k", 2)],
                4: [vu(8), vu(9)],
                5: [vu(10), vu(11), qk(0, "k", 3)],
                6: [vu(12), vu(13)],
                7: [vu(14), vu(15)],
            })
            attention_chunk(0, 1, fills={
                1: [qk(0, "q", 1)],
                5: [qk(0, "q", 2)],
            })
            attention_chunk(0, 2, fills={3: [qk(1, "k", 0)]})
            attention_chunk(0, 3, fills={3: [qk(1, "k", 1)]})
            attention_chunk(0, 4, fills={1: [qk(0, "q", 3)],
                                         5: [qk(1, "k", 2)]})
            attention_chunk(0, 5, fills={3: [qk(1, "k", 3)]})
            attention_chunk(0, 6, fills={3: [qk(1, "q", 0)]})
            attention_chunk(0, 7, fills={3: [qk(1, "q", 1)]})
            attention_chunk(1, 0, fills={})
            attention_chunk(1, 1, fills={
                0: [fc(0, 0, "scalar")], 2: [fc(0, 1, "vector")],
                4: [fc(1, 0, "scalar")], 6: [fc(1, 1, "vector")],
                3: [qk(1, "q", 2)],
            })
            for qc2 in range(2, 7):
                fmts = (2 * (qc2 - 1), 2 * (qc2 - 1) + 1)
                fills = {
                    0: [fc(fmts[0], 0, "scalar")],
                    2: [fc(fmts[0], 1, "vector")],
                    4: [fc(fmts[1], 0, "scalar")],
                    6: [fc(fmts[1], 1, "vector")],
                }
                if qc2 == 3:
                    fills[3] = [qk(1, "q", 3)]
                attention_chunk(1, qc2, fills=fills)
            attention_chunk(1, 7, qw=128, qcol=1792, fills={
                0: [fc(12, 0, "scalar")], 2: [fc(12, 1, "vector")],
                4: [fc(13, 0, "scalar")], 6: [fc(13, 1, "vector")],
            })
            attention_chunk(1, 7, qw=128, qcol=1920, fills={
                2: [fc(14, 0, "scalar")], 5: [fc(14, 1, "vector")],
            })
            fc_unit(15, 0, "scalar")
            fc_unit(15, 1, "vector")

    nc.compile()
    return nc


def make_in_maps(x, Wq, bq, Wk, bk, Wv, bv, Wo, bo):
    import ml_dtypes

    f = np.float32
    bf = ml_dtypes.bfloat16
    in_maps = []
    for core in range(N_CORES):
        b, g = divmod(core, GROUPS)
        cs = slice(g * C, (g + 1) * C)
        wq = np.asarray(Wq[:, cs], f) * np.float32(PRESCALE)
        bqc = np.asarray(bq[cs], f) * np.float32(PRESCALE)
        m = {
            "xT": np.ascontiguousarray(np.asarray(x[b]).T.astype(bf)),
            "wq": np.ascontiguousarray(wq.astype(bf)),
            "wk": np.ascontiguousarray(np.asarray(Wk[:, cs], f).astype(bf)),
            "wv": np.ascontiguousarray(np.asarray(Wv[:, cs], f).astype(bf)),
            "wo": np.ascontiguousarray(np.asarray(Wo[cs], f).astype(bf)),
            "bqs": np.ascontiguousarray(bqc.reshape(MC, 128).T),
            "bks": np.ascontiguousarray(
                np.asarray(bk[cs], f).reshape(MC, 128).T),
            "bvb": np.ascontiguousarray(np.broadcast_to(bv[cs], (128, C)),
                                        dtype=f),
        }
        in_maps.append(m)
    return in_maps


_nc_cache = {}


def kernel(x, Wq, bq, Wk, bk, Wv, bv, Wo, bo, _trace=False):
    from concourse.bass_utils import run_bass_kernel_spmd

    if "nc" not in _nc_cache:
        _nc_cache["nc"] = build_nc()
    nc = _nc_cache["nc"]
    in_maps = make_in_maps(x, Wq, bq, Wk, bk, Wv, bv, Wo, bo)
    res = run_bass_kernel_spmd(nc, in_maps, core_ids=list(range(N_CORES)),
                               trace=_trace)
    _nc_cache["last_result"] = res
    out = np.empty((B, S, D), np.float32)
    bo_f = np.asarray(bo, np.float32)
    for b in range(B):
        acc = np.asarray(res.results[b * GROUPS]["out"], np.float32)
        for g in range(1, GROUPS):
            acc = acc + np.asarray(res.results[b * GROUPS + g]["out"],
                                   np.float32)
        out[b] = acc + bo_f
    return out
